# revision 4
# baseline (speedup 1.0000x reference)
"""GCN layer (GCNConv forward) on 8 Trainium2 NeuronCores — v7 (asymmetric banks: small bank 0 shortens the phase-1 head).

out = D^-1/2 (A+I) D^-1/2 (x @ W) + b   with random edge_index [2, E].

v2 changes vs baseline:
  - y table stored bf16 as pair-tokens: y2[k, 0:64] = y[2k], y2[k, 64:128] =
    y[2k+1]; gather elem = 256B (128 bf16) with idx = row//2 per 65536-row
    bank (2 banks) -> no f32->bf16 cast op, half the y write traffic
  - edges presorted by (bank, dest-tile, parity-of-row); per-subsegment quota
    rounded to 32 (not 128) -> ~1.02x padding instead of 1.30x; 128-edge
    chunks may cross one subsegment boundary, handled by a 256-wide iota
    is_equal + 2 matmuls on those chunks only
  - y banks are separate dram tensors so bank-0 gather overlaps bank-1 xw
"""
import os
import sys

if "/opt/trn_rl_repo" not in sys.path:
    sys.path.insert(0, "/opt/trn_rl_repo")

import numpy as np
import ml_dtypes
from contextlib import ExitStack

import concourse.bacc as bacc
import concourse.bass as bass
import concourse.mybir as mybir
import concourse.tile as tile
from concourse import library_config
from concourse._compat import cdiv
from concourse.bass_utils import run_bass_kernel_spmd

# ---------------- problem constants (hardcoded per spec) ----------------
N = 100000
E = 1600000
C = 64
NCORES = 8
NSHARD = N // NCORES            # 12500 dest rows per core
P = 128
NT = cdiv(NSHARD, P)            # 98 dest tiles per core
B0ROWS = 34816                  # bank0 rows (17408 pairs); bank1 = 65536 rows
NBANK = 2
CALL = int(os.environ.get("GCN_CALL", "1024"))
NQ = int(os.environ.get("GCN_QUEUES", "2"))
SCOFF = int(os.environ.get("GCN_SCOFF", "0"))  # every SCOFF-th chunk on Scalar; 0=off
WIN = 512                       # xw phase: nodes per y-write window (wrap-4)
WRAP = 4
XT_BLOCK = 12288
N_PAD = 100352                  # multiple of 512; pair rows = 50176
NPAIR = N_PAD // 2
NU = N_PAD // P                 # 784 dinv columns
QROUND = 16

BF16 = ml_dtypes.bfloat16
NSUB = NBANK * NT * 2           # subsegments: (bank, tile, parity)


def _wrap4_node_index():
    p = np.arange(P)[:, None]
    u = np.arange(NU)[None, :]
    return (u // WRAP) * WIN + p * WRAP + (u % WRAP)


# ---------------- host-side preprocessing ----------------
def preprocess(x, edge_index, W, b):
    x = np.asarray(x, np.float32)
    edge_index = np.asarray(edge_index)
    W = np.asarray(W, np.float32)
    b = np.asarray(b, np.float32)
    row = edge_index[0].astype(np.int64)
    col = edge_index[1].astype(np.int64)

    cnt = np.bincount(col, minlength=N).astype(np.int64)
    rowptr = np.concatenate([[0], np.cumsum(cnt)])

    loops = np.arange(N, dtype=np.int64)
    row = np.concatenate([row, loops])
    col = np.concatenate([col, loops])

    shard = col // NSHARD
    per_core = []
    counts = np.zeros((NCORES, NSUB), np.int64)
    for c in range(NCORES):
        m = shard == c
        r = row[m]
        cl = col[m] - c * NSHARD
        # subsegment: bank-major, then dest tile, then source-row parity
        bkv = (r >= B0ROWS).astype(np.int64)
        g = (bkv * NT + cl // P) * 2 + (r % 2)
        order = np.argsort(g, kind="stable")
        r, cl, g = r[order], cl[order], g[order]
        counts[c] = np.bincount(g, minlength=NSUB)
        per_core.append((r, cl, g))

    quota = (np.ceil(counts.max(axis=0) / QROUND).astype(np.int64)) * QROUND
    quota = np.maximum(quota, P)     # chunk spans <=2 subsegments
    # bank totals must be multiples of 128 (call/chunk alignment)
    for bk in range(NBANK):
        lo, hi = bk * NT * 2, (bk + 1) * NT * 2
        rem = quota[lo:hi].sum() % P
        if rem:
            quota[hi - 1] += P - rem
    qoff = np.concatenate([[0], np.cumsum(quota)])
    total = int(qoff[-1])
    assert total % P == 0

    bank_len = [int(quota[bk * NT * 2:(bk + 1) * NT * 2].sum())
                for bk in range(NBANK)]
    bank_off = np.concatenate([[0], np.cumsum(bank_len)]).astype(np.int64)
    calls = []                            # (bank, stream_start, n_idx)
    for bk in range(NBANK):
        s = int(bank_off[bk])
        while s < int(bank_off[bk + 1]):
            n = min(CALL, int(bank_off[bk + 1]) - s)
            calls.append((bk, s, n))
            s += n

    # per-chunk metadata (identical across cores): for chunk k, the
    # subsegment of its first position and whether it crosses into the next
    nchunk = total // P
    pos0 = np.arange(nchunk) * P
    sub0 = np.searchsorted(qoff, pos0, side="right") - 1
    subL = np.searchsorted(qoff, pos0 + P - 1, side="right") - 1
    assert (subL - sub0 <= 1).all(), "chunk spans >2 subsegments"
    chunk_sub0 = sub0.astype(np.int64)
    chunk_cross = (subL > sub0)

    struct = {"quota": quota.tolist(), "qoff": qoff.tolist(), "total": total,
              "calls": calls, "chunk_sub0": chunk_sub0.tolist(),
              "chunk_cross": chunk_cross.tolist(), "bank_off": bank_off.tolist()}

    # ---- shared arrays ----
    S16, S128 = total // 16, total // 128
    xT = np.zeros((C, N_PAD), np.float32)
    xT[:, :N] = x.T
    xT = np.ascontiguousarray(xT.astype(BF16))
    W_bf = np.ascontiguousarray(np.tile(W, (2, 1)).astype(BF16))
    b_bcast = np.ascontiguousarray(np.tile(b[None, :], (P, 1)).astype(np.float32))

    nid = _wrap4_node_index()
    valid = nid < N
    rpA = np.zeros((P, NU), np.float32)
    rpB = np.zeros((P, NU), np.float32)
    rpA[valid] = rowptr[nid[valid]]
    rpB[valid] = rowptr[nid[valid] + 1]

    in_maps = []
    for c in range(NCORES):
        r, cl, g = per_core[c]
        cnt_c = counts[c]
        gstart = np.concatenate([[0], np.cumsum(cnt_c)])
        rank = np.arange(len(g)) - gstart[g]
        pos = qoff[g] + rank

        idx_rel = np.zeros(total, np.int64)            # pads gather token 0
        val = np.full(total, 999.0, np.float32)        # pads never match iota
        idx_rel[pos] = (r - (r >= B0ROWS) * B0ROWS) // 2
        # d within tile, plus 128 if edge belongs to chunk's second subsegment
        d = cl % P
        edge_chunk = pos // P
        off_code = g - chunk_sub0[edge_chunk]
        assert ((off_code == 0) | (off_code == 1)).all()
        val[pos] = d + 128 * off_code

        idx16 = np.zeros((16, S16), np.int16)
        idx16[np.arange(total) % 16, np.arange(total) // 16] = idx_rel
        idx16 = np.ascontiguousarray(np.tile(idx16, (8, 1)))

        valw = np.zeros((P, S128), np.float32)
        valw[np.arange(total) % P, np.arange(total) // P] = val

        pp = np.arange(P)[:, None]
        tt = np.arange(NT)[None, :]
        nd = c * NSHARD + tt * P + pp
        vd = nd < N
        rpdA = np.zeros((P, NT), np.float32)
        rpdB = np.zeros((P, NT), np.float32)
        rpdA[vd] = rowptr[nd[vd]]
        rpdB[vd] = rowptr[nd[vd] + 1]

        in_maps.append({
            "xT": xT, "W": W_bf, "bb": b_bcast, "rpA": rpA, "rpB": rpB,
            "rpdA": np.ascontiguousarray(rpdA),
            "rpdB": np.ascontiguousarray(rpdB),
            "idx16": idx16, "valw": np.ascontiguousarray(valw),
        })
    return in_maps, struct


# ---------------- device program ----------------
def build_program(struct):
    quota = struct["quota"]
    qoff = struct["qoff"]
    total = struct["total"]
    all_calls = struct["calls"]
    chunk_sub0 = struct["chunk_sub0"]
    chunk_cross = struct["chunk_cross"]
    bank_off = struct["bank_off"]
    S16, S128 = total // 16, total // 128
    phases = os.environ.get("GCN_PHASES", "123")
    skip = os.environ.get("GCN_SKIP", "")
    rep = int(os.environ.get("GCN_REPEAT", "1"))
    maxcalls = int(os.environ.get("GCN_MAXCALLS", "1000000"))

    nc = bacc.Bacc("TRN2", target_bir_lowering=False, debug=True,
                   dynamic_dma_scratch_size=16 * CALL * NQ,
                   num_swdge_queues=NQ)
    f32, bf16, i16 = mybir.dt.float32, mybir.dt.bfloat16, mybir.dt.int16

    xT_d = nc.dram_tensor("xT", [C, N_PAD], bf16, kind="ExternalInput")
    W_d = nc.dram_tensor("W", [2 * C, C], bf16, kind="ExternalInput")
    bb_d = nc.dram_tensor("bb", [P, C], f32, kind="ExternalInput")
    rpA_d = nc.dram_tensor("rpA", [P, NU], f32, kind="ExternalInput")
    rpB_d = nc.dram_tensor("rpB", [P, NU], f32, kind="ExternalInput")
    rpdA_d = nc.dram_tensor("rpdA", [P, NT], f32, kind="ExternalInput")
    rpdB_d = nc.dram_tensor("rpdB", [P, NT], f32, kind="ExternalInput")
    idx_d = nc.dram_tensor("idx16", [P, S16], i16, kind="ExternalInput")
    val_d = nc.dram_tensor("valw", [P, S128], f32, kind="ExternalInput")
    out_d = nc.dram_tensor("out", [P, NT, C], f32, kind="ExternalOutput")
    ybank_d = [nc.dram_tensor("yb0", [B0ROWS // 2, 2 * C], bf16,
                              kind="Internal"),
               nc.dram_tensor("yb1", [NPAIR - B0ROWS // 2, 2 * C], bf16,
                              kind="Internal")]

    with tile.TileContext(nc) as tc:
        with ExitStack() as ctx:
            const = ctx.enter_context(tc.tile_pool(name="const", bufs=1))
            psum_pool = ctx.enter_context(
                tc.tile_pool(name="psum", bufs=8, space="PSUM"))
            dtmp = ctx.enter_context(tc.tile_pool(name="dtmp", bufs=1))
            xtp = ctx.enter_context(tc.tile_pool(name="xt", bufs=2))
            ysbp = ctx.enter_context(tc.tile_pool(name="ysb", bufs=4))
            gbp = ctx.enter_context(tc.tile_pool(name="gb", bufs=6))
            indp = ctx.enter_context(tc.tile_pool(name="ind", bufs=8))

            nc.gpsimd.load_library(library_config.mlp)

            W_sb = const.tile([2 * C, C], bf16, tag="W")
            bb_sb = const.tile([P, C], f32, tag="bb")
            iota_i = const.tile([P, 2 * P], i16, tag="iota_i")
            iota_bf = const.tile([P, 2 * P], bf16, tag="iota_bf")
            dinv_g = const.tile([P, NU], f32, tag="dinv_g")
            dinv_d = const.tile([P, NT], f32, tag="dinv_d")
            acc = const.tile([P, NT * C], f32, tag="acc")
            idx_sb = const.tile([P, S16], i16, tag="idx")
            val_sb = const.tile([P, S128], f32, tag="valw")
            negval_sb = const.tile([P, S128], f32, tag="negval")

            nc.sync.dma_start(W_sb[:], W_d[:])
            nc.sync.dma_start(bb_sb[:], bb_d[:])
            nc.sync.dma_start(idx_sb[:], idx_d[:])
            nc.sync.dma_start(val_sb[:], val_d[:])
            nc.gpsimd.iota(iota_i[:], pattern=[[1, 2 * P]], channel_multiplier=0)
            nc.vector.memset(acc[:], 0.0)
            nc.vector.tensor_copy(iota_bf[:], iota_i[:])
            nc.vector.tensor_scalar_mul(negval_sb[:], val_sb[:], -1.0)

            def emit_body():
                # ---- dinv = sqrt(1 / (rowptr[n+1]-rowptr[n]+1)) ----
                for (ad, bd, w, dst) in ((rpA_d, rpB_d, NU, dinv_g),
                                         (rpdA_d, rpdB_d, NT, dinv_d)):
                    ta = dtmp.tile([P, NU], f32, tag="ta", name="ta")
                    tb = dtmp.tile([P, NU], f32, tag="tb", name="tb")
                    nc.sync.dma_start(ta[:, :w], ad[:])
                    nc.sync.dma_start(tb[:, :w], bd[:])
                    nc.vector.tensor_tensor(tb[:, :w], tb[:, :w], ta[:, :w],
                                            mybir.AluOpType.subtract)
                    nc.vector.tensor_scalar_add(tb[:, :w], tb[:, :w], 1.0)
                    nc.vector.reciprocal(ta[:, :w], tb[:, :w])
                    nc.scalar.activation(dst[:], ta[:, :w],
                                         mybir.ActivationFunctionType.Sqrt)

                # ---- phase 1: y = dinv * (x @ W), bf16 pair-token layout ----
                blocks = []
                base = 0
                while base < N_PAD and "1" in phases:
                    nblk = min(XT_BLOCK, N_PAD - base)
                    blocks.append((base, nblk))
                    base += nblk
                for (base, nblk) in blocks:
                    half = nblk // 2
                    xt = xtp.tile([P, XT_BLOCK // 2], bf16, tag="xt", name="xt")
                    src = bass.AP(xT_d, base,
                                  [[half, 2], [N_PAD, C], [1, half]])
                    nc.sync.dma_start(xt[:, :half], src)
                    for w in range(nblk // WIN):
                        wbase = base + w * WIN
                        h = (w * WIN) // half
                        foff = (w * WIN) % half
                        ysb = ysbp.tile([P, WRAP, C], bf16, tag="ysb",
                                        name="ysb")
                        u0 = (wbase // WIN) * WRAP
                        for s in range(WRAP):
                            ps = psum_pool.tile([P, C], f32, tag="mm",
                                                name="mmps")
                            lhsT = xt[h * C:(h + 1) * C,
                                      foff + s: foff + s + WRAP * (P - 1) + 1: WRAP]
                            nc.tensor.matmul(ps[:], lhsT,
                                             W_sb[h * C:(h + 1) * C, :],
                                             start=True, stop=True)
                            if s == 0:
                                nc.scalar.activation(
                                    ysb[:, s, :], ps[:],
                                    mybir.ActivationFunctionType.Copy,
                                    scale=dinv_g[:, u0 + s: u0 + s + 1])
                            else:
                                nc.vector.tensor_scalar_mul(
                                    ysb[:, s, :], ps[:],
                                    dinv_g[:, u0 + s: u0 + s + 1])
                        # partition k holds pair rows (wbase/2 + 2k, +2k+1)
                        if wbase < B0ROWS:
                            yb, pr = ybank_d[0], wbase // 2
                        else:
                            yb, pr = ybank_d[1], (wbase - B0ROWS) // 2
                        dst = bass.AP(yb, pr * 2 * C,
                                      [[2 * 2 * C, P], [2 * C, 2], [1, 2 * C]])
                        nc.sync.dma_start(dst, ysb[:])

                # ---- phase 2: gather pair-tokens + indicator matmuls ----
                calls = all_calls if "2" in phases else []
                calls = calls[:maxcalls]

                def sub_info(sub):
                    bk, rest = divmod(sub, NT * 2)
                    t, par = divmod(rest, 2)
                    return bk, t, par

                # first/last chunk-matmul indices per (bank, tile) session
                tile_first = {}
                tile_last = {}
                for k in range(total // P):
                    subs = [chunk_sub0[k]] + (
                        [chunk_sub0[k] + 1] if chunk_cross[k] else [])
                    for sub in subs:
                        bk, t, par = sub_info(sub)
                        key = (bk, t)
                        if key not in tile_first:
                            tile_first[key] = (k, sub)
                        tile_last[key] = (k, sub)

                psum_by_tile = {}
                for ci, (bk, cstart, cn) in enumerate(calls):
                    gbuf = gbp.tile([P, CALL // P, 2 * C], bf16, tag="gbuf",
                                    name="gbuf")
                    nslots = cn // P
                    bank_pairs = (B0ROWS // 2 if bk == 0
                                  else NPAIR - B0ROWS // 2)
                    if "g" not in skip:
                        nc.gpsimd.dma_gather(
                            gbuf[:, :nslots, :],
                            ybank_d[bk][:bank_pairs, :],
                            idx_sb[:, cstart // 16: (cstart + cn) // 16],
                            cn, cn, 2 * C, queue_num=ci % NQ)

                    for k in range(cstart // P, (cstart + cn) // P):
                        slot = k - cstart // P
                        wid = 2 * P if chunk_cross[k] else P
                        ind = indp.tile([P, 2 * P], bf16, tag="ind",
                                        name="ind")
                        if "i" in skip:
                            nc.scalar.activation(
                                ind[:, :wid], iota_bf[:, :wid],
                                mybir.ActivationFunctionType.Copy)
                        elif SCOFF and k % SCOFF == SCOFF - 1:
                            tmp = indp.tile([P, 2 * P], bf16, tag="ind",
                                            name="indtmp")
                            nc.scalar.activation(
                                tmp[:, :wid], iota_bf[:, :wid],
                                mybir.ActivationFunctionType.Abs,
                                bias=negval_sb[:, k: k + 1])
                            nc.scalar.activation(
                                ind[:, :wid], tmp[:, :wid],
                                mybir.ActivationFunctionType.Relu,
                                scale=-1.0, bias=1.0)
                        else:
                            nc.vector.tensor_scalar(
                                ind[:, :wid], iota_bf[:, :wid],
                                val_sb[:, k: k + 1], None,
                                mybir.AluOpType.is_equal)
                        subs = [chunk_sub0[k]] + (
                            [chunk_sub0[k] + 1] if chunk_cross[k] else [])
                        if "m" in skip:
                            subs = []
                        for j, sub in enumerate(subs):
                            sbk, t, par = sub_info(sub)
                            key = (sbk, t)
                            if tile_first[key] == (k, sub):
                                psum_by_tile[t] = psum_pool.tile(
                                    [P, C], f32, tag="mm",
                                    name=f"pst_b{sbk}_t{t}")
                            ps = psum_by_tile[t]
                            nc.tensor.matmul(
                                ps[:], ind[:, j * P:(j + 1) * P],
                                gbuf[:, slot, par * C:(par + 1) * C],
                                start=(tile_first[key] == (k, sub)),
                                stop=(tile_last[key] == (k, sub)))
                            if tile_last[key] == (k, sub):
                                a = acc[:, t * C:(t + 1) * C]
                                if sbk == 0:
                                    nc.vector.tensor_copy(a, ps[:])
                                else:
                                    nc.vector.tensor_tensor(
                                        a, a, ps[:], mybir.AluOpType.add)
                                    nc.vector.tensor_scalar_mul(
                                        a, a, dinv_d[:, t: t + 1])
                                    nc.vector.tensor_tensor(
                                        a, a, bb_sb[:], mybir.AluOpType.add)
                                    nc.sync.dma_start(
                                        out_d[:, t, :], a)
                                del psum_by_tile[t]

                for t, ps in list(psum_by_tile.items()):
                    # truncated-call debug runs leave open groups; close them
                    a = acc[:, t * C:(t + 1) * C]
                    nc.vector.tensor_copy(a, ps[:])
                    nc.vector.tensor_scalar_mul(a, a, dinv_d[:, t: t + 1])
                    nc.vector.tensor_tensor(a, a, bb_sb[:], mybir.AluOpType.add)
                    nc.sync.dma_start(out_d[:, t, :], a)
                    del psum_by_tile[t]
                if "2" not in phases or not calls:
                    # no phase 2 at all: still produce the output tensor
                    nc.sync.dma_start(
                        out_d[:], acc[:].rearrange("p (t c) -> p t c", c=C))

            if rep > 1:
                with tc.For_i(0, rep, 1):
                    emit_body()
            else:
                emit_body()

    nc.compile()
    return nc


# ---------------- entry point ----------------
_CACHE = {}


def kernel(x, edge_index, W, b):
    in_maps, struct = preprocess(x, edge_index, W, b)
    key = (struct["total"], tuple(struct["quota"]))
    if key not in _CACHE:
        _CACHE.clear()
        _CACHE[key] = build_program(struct)
    nc = _CACHE[key]
    res = run_bass_kernel_spmd(nc, in_maps, core_ids=list(range(NCORES)))
    outs = []
    for c in range(NCORES):
        o = res.results[c]["out"]                      # [P, NT, C]
        o = np.transpose(o, (1, 0, 2)).reshape(NT * P, C)[:NSHARD]
        outs.append(o)
    return np.concatenate(outs, axis=0).astype(np.float32)
